# revision 34
# baseline (speedup 1.0000x reference)
"""AttentionClustering (vq_codebook) Trainium2 kernel, 8-core data parallel.

Shard: 8 cores = 4 images x 2 half-images (128 output rows each). Odd cores
get a vertically flipped shard + row-flipped conv weights so every core's
program is identical (true image edge at local top, interior halo at bottom).

Math: q1 = relu(conv3x3(x, w1) + b1); q2 = relu(conv3x3(q1, w2) + b2)  (both
with replicate padding); then the 1x1 conv + cluster-distance softmax folds to
  logit[px, k] = sum_ci q2[ci, px] * muW[k, ci] + bp[k]
  muW = 2 * mu @ W3,  bp = 2 * mu @ b3 - |mu|^2      (|q|^2 cancels in softmax)
  out[px] = sum_k softmax_k(logit) * label[k]

conv2 (the FLOP-dominant layer) runs as Winograd F(2,3) along H: for each
output row pair, V_p = B^T [q1 rows 2i-1..2i+2] (4 DVE adds per 4 tile-rows),
m_p = sum_{kc,dc} Gw2[p,kc,dc] @ V_p[:, dc:dc+256] (24 matmuls of N=512 per
i-PAIR per mc vs 36 for direct conv -> 1.5x fewer PE columns), then
out_even = m0+m1+m2, out_odd = m1-m2-m3 (4 DVE adds) + ACT relu. conv1 stays
direct (dual-tap packed, near-ideal).

All matmuls run in fp16: full PE rate at any N, fp16 LDWEIGHTS hides under the
N=512 streams. fp32 accumulate in PSUM.
"""
import sys
if '/opt/trn_rl_repo' not in sys.path:
    sys.path.insert(0, '/opt/trn_rl_repo')

import numpy as np
import concourse.bass as bass
import concourse.mybir as mybir
from concourse import bacc, tile
from concourse.bass_utils import run_bass_kernel_spmd

F32 = mybir.dt.float32
F16 = mybir.dt.float16
F8 = mybir.dt.float8e3
AF = mybir.ActivationFunctionType
ALU = mybir.AluOpType
AX = mybir.AxisListType

B, CIN, H, W = 4, 64, 256, 256
Q, K = 256, 16
RB = 32           # output rows per band
NBAND = 4         # bands per core (128 rows)
NCORES = 8

_cached = {}


def build_nc():
    nc = bacc.Bacc("TRN2", target_bir_lowering=False, debug=False)

    CHS = 132 * (W + 2)          # per-channel element stride in flat xh
    xh = nc.declare_dram_parameter("xh", [CIN * CHS + 2 * (W + 2)], F16,
                                   isOutput=False)
    w1x = nc.declare_dram_parameter("w1x", [128, 8, 128], F16, isOutput=False)
    w1s = nc.declare_dram_parameter("w1s", [64, 2, 128], F16, isOutput=False)
    w2l = nc.declare_dram_parameter("w2l", [128, 72, 128], F16, isOutput=False)
    muw = nc.declare_dram_parameter("muw", [128, 2, K], F16, isOutput=False)
    cst = nc.declare_dram_parameter("cst", [128, 2 * K + 4], F32, isOutput=False)
    # out stays in the on-chip [px-partition, band, g, r, jj] layout; the host
    # unscrambles. A row-major dram layout would need 4 B DMA descriptors.
    outd = nc.declare_dram_parameter("out", [128, NBAND * RB * 2], F32,
                                     isOutput=True)

    with tile.TileContext(nc) as tc:
        with tc.tile_pool(name="singles", bufs=1) as singles, \
             tc.tile_pool(name="xpool", bufs=1) as xpool, \
             tc.tile_pool(name="q1pool", bufs=1) as q1pool, \
             tc.tile_pool(name="vpool", bufs=2) as vpool, \
             tc.tile_pool(name="vscr", bufs=8) as vscr, \
             tc.tile_pool(name="tupool", bufs=2) as tupool, \
             tc.tile_pool(name="q2pre", bufs=1) as q2prep, \
             tc.tile_pool(name="q2pool", bufs=2) as q2pool, \
             tc.tile_pool(name="smx", bufs=1) as smx, \
             tc.tile_pool(name="obuf", bufs=2) as obuf:

            # ---- resident weights (one DMA per family) ----------------
            w1xbuf = singles.tile([128, 8, 128], F16, tag="w1xbuf")
            nc.sync.dma_start(out=w1xbuf, in_=w1x.ap())
            w1a_sb = {(mc, dr): w1xbuf[:, mc * 3 + dr, :]
                      for mc in range(2) for dr in range(3)}
            w1r_sb = {mc: w1xbuf[:, 6 + mc, :] for mc in range(2)}
            w1sbuf = singles.tile([64, 2, 128], F16, tag="w1sbuf")
            nc.sync.dma_start(out=w1sbuf, in_=w1s.ap())
            w1s_sb = {mc: w1sbuf[:, mc, :] for mc in range(2)}
            # band-0 x halo first so conv1 can start before w2 finishes loading
            xh_ap = xh.ap()

            def xsrc(r0, lo, hi, shift):
                # [64ch, elems] slice of the flat xh, shifted by `shift`
                # elements (1 = one column, W+2 = one row). Rows are contiguous
                # in dram, so one big run per channel keeps the DMA
                # descriptor count at 64/transfer instead of 64*rows.
                return bass.AP(
                    tensor=xh_ap.tensor,
                    offset=(r0 + lo) * (W + 2) + shift,
                    ap=[[CHS, CIN], [1, (hi - lo) * (W + 2)]])

            def load_xband(r0, rb):
                chunks = [(0, rb + 4)]
                xa = xpool.tile([128, rb + 4, W + 2], F16, tag="xa", name="xa", bufs=2)
                xr = xpool.tile([128, rb + 4, W + 2], F16, tag="xr", name="xr")
                xaf = xa.rearrange("p r c -> p (r c)")
                xrf = xr.rearrange("p r c -> p (r c)")
                for lo, hi in chunks:
                    cl, ch = lo * (W + 2), hi * (W + 2)
                    nc.sync.dma_start(out=xaf[0:64, cl:ch],
                                      in_=xsrc(r0, lo, hi, 0))
                    nc.sync.dma_start(out=xaf[64:128, cl:ch],
                                      in_=xsrc(r0, lo, hi, 1))
                    nc.sync.dma_start(out=xrf[0:64, cl:ch],
                                      in_=xsrc(r0, lo, hi, 0))
                    nc.sync.dma_start(out=xrf[64:128, cl:ch],
                                      in_=xsrc(r0, lo, hi, W + 2))
                return xa, xr

            xband0 = load_xband(0, 16)

            # small constants next (before the bulky w2 tiles hog the queues)
            muwbuf = singles.tile([128, 2, K], F16, tag="muwbuf")
            nc.sync.dma_start(out=muwbuf, in_=muw.ap())
            muw_sb = {kc: muwbuf[:, kc, :] for kc in range(2)}
            cstbuf = singles.tile([128, 2 * K + 4], F32, tag="cstbuf")
            nc.sync.dma_start(out=cstbuf, in_=cst.ap())
            bp_sb = cstbuf[:, 0:K]
            lab_sb = cstbuf[:, K:2 * K]
            cb16 = singles.tile([128, 4], F16, tag="cb16")
            nc.vector.tensor_copy(cb16, cstbuf[:, 2 * K:2 * K + 4])
            b1_sb = {mc: cb16[:, mc:mc + 1] for mc in range(2)}
            b2_sb = {mc: cb16[:, 2 + mc:3 + mc] for mc in range(2)}

            # PE warmup: keep TensorE busy through the initial DMA wait so
            # the HAM clock-gate is at 8/8 when real matmuls arrive.
            wscr = singles.tile([128, 512], F16, tag="wscr")
            nc.vector.memset(wscr, 0.0)
            with tc.tile_pool(name="psw", bufs=1, space="PSUM") as psw:
                wps = psw.tile([128, 512], F32, tag="wps", name="wps")
                for _ in range(46):
                    nc.tensor.matmul(wps, wscr[:, 0:128], wscr,
                                     start=True, stop=True)

            w2buf = singles.tile([128, 72, 128], F16, tag="w2buf")
            nc.sync.dma_start(out=w2buf[:, 0:24, :], in_=w2l.ap()[:, 0:24, :])
            nc.sync.dma_start(out=w2buf[:, 24:72, :], in_=w2l.ap()[:, 24:72, :])
            # Winograd-transformed conv2 weights: index (pos, kc, dc, mc)
            w2_sb = {(p, kc, dc, mc): w2buf[:, ((p * 2 + kc) * 3 + dc) * 2 + mc, :]
                     for p in range(6) for kc in range(2)
                     for dc in range(3) for mc in range(2)}

            with tc.tile_pool(name="ps1", bufs=2, space="PSUM") as ps1, \
                 tc.tile_pool(name="ps2", bufs=5, space="PSUM") as ps2, \
                 tc.tile_pool(name="psl", bufs=1, space="PSUM") as psl:

                # Logits matmuls of ipair p are deferred and interleaved
                # into the next ipair's conv2 stream: their ~97ns LDWEIGHTS
                # then hides under the N=512 conv2 streams (the LDW port has
                # 216-2*97 ns of slack per conv2 MM) instead of serializing.
                pend = {'mms': [], 'fin': None}

                def emit_pending_mms(k):
                    while k > 0 and pend['mms']:
                        pend['mms'].pop(0)()
                        k -= 1

                def flush_pending():
                    while pend['mms']:
                        pend['mms'].pop(0)()
                    if pend['fin'] is not None:
                        pend['fin']()
                        pend['fin'] = None

                # ---- bands: band 0 split in half so the first x halo
                # load (and hence conv1) is ready sooner ------------------
                BANDS = [(0, 16), (16, 16), (32, 32), (64, 32), (96, 16), (112, 16)]
                for bi, (r0, rb) in enumerate(BANDS):
                    # x halo in two packings:
                    #  xa: p0-63 = xh rows, p64-127 = same shifted +1 col
                    #  xr: p0-63 = xh rows, p64-127 = same shifted +1 row
                    xa, xr = xband0 if bi == 0 else load_xband(r0, rb)

                    # q1 band buffer: slot j = q1 row (r0 - 1 + j), cols
                    # 1..256 real, cols 0/257 replicate pads.
                    q1b = {}
                    nslot = ((rb + 2 + 3) // 4) * 4
                    for kc in range(2):
                        q1b[kc] = q1pool.tile([128, nslot, W + 2], F16,
                                              tag=f"q1_{kc}", name=f"q1_{kc}")
                    q1v = {kc: q1b[kc].rearrange("p (s four) c -> p s four c", four=4)
                           for kc in range(2)}

                    # conv1: q1 slot j needs xh local rows j+dr (pairs), and
                    # taps (0,2),(1,2) from xr row j, tap (2,2) from xa row j+2.
                    nch = rb // 8
                    if bi == 0:
                        groups1 = [(j, 2) for j in range(1, rb + 1, 2)] + [(rb + 1, 1)]
                        ready_j = {c: 8 * c + 10 for c in range(nch - 1)}
                        ready_j[nch - 1] = rb + 1
                    else:
                        groups1 = [(j, 2) for j in range(0, rb + 2, 2)]
                        ready_j = {c: 8 * c + 9 for c in range(nch)}
                    j_to_chunk = {v: k for k, v in ready_j.items()}

                    vtiles = {}

                    def emit_vchunk(c):
                        # replicate-pad cols for the slot range this chunk reads
                        lo = (1 if bi == 0 else 0) if c == 0 else 8 * c + 2
                        hi = 8 * c + 10          # exclusive
                        for kc in range(2):
                            nc.vector.tensor_copy(
                                out=q1b[kc][:, lo:hi, 0:1],
                                in_=q1b[kc][:, lo:hi, 1:2])
                            nc.vector.tensor_copy(
                                out=q1b[kc][:, lo:hi, W + 1:W + 2],
                                in_=q1b[kc][:, lo:hi, W:W + 1])
                        if bi == 0 and c == 0:
                            for kc in range(2):
                                nc.vector.tensor_copy(
                                    out=q1b[kc][:, 0:1, :], in_=q1b[kc][:, 1:2, :])
                        # B^T row-transform (F(4,3)): 6 V planes for the two
                        # 4-row tiles i4 in {2c, 2c+1}; d_j = slot 4*i4 + j.
                        #  v0 = (d4-d2) - 4(d2-d0)      v1 = (d3+d4) - 4(d1+d2)
                        #  v2 = (d4-d3) + 4(d1-d2)      v3 = (d4-d2) + 2(d3-d1)
                        #  v4 = (d4-d2) - 2(d3-d1)      v5 = (d5-d3) - 4(d3-d1)
                        vts = {}
                        for kc in range(2):
                            dj = [q1v[kc][:, 2 * c + (j // 4):2 * c + (j // 4) + 2,
                                          j % 4, :] for j in range(6)]
                            vt = vpool.tile([128, 2, 6, W + 2], F16,
                                            tag=f"v{kc}", name=f"v{kc}")

                            def vs():
                                return vscr.tile([128, 2, W + 2], F16,
                                                 tag="vs", name="vs", bufs=8)
                            TT, TS = nc.vector.tensor_tensor, nc.vector.tensor_scalar_mul
                            # planes emitted in conv2 consumption order (v0..v5)
                            b_ = vs(); TT(b_, dj[4], dj[2], ALU.subtract)
                            h_ = vs(); TT(h_, dj[2], dj[0], ALU.subtract)
                            h4 = vs(); TS(h4, h_, 4.0)
                            TT(vt[:, :, 0, :], b_, h4, ALU.subtract)
                            f_ = vs(); TT(f_, dj[1], dj[2], ALU.add)
                            f4 = vs(); TS(f4, f_, 4.0)
                            g_ = vs(); TT(g_, dj[3], dj[4], ALU.add)
                            TT(vt[:, :, 1, :], g_, f4, ALU.subtract)
                            c_ = vs(); TT(c_, dj[1], dj[2], ALU.subtract)
                            c4 = vs(); TS(c4, c_, 4.0)
                            e_ = vs(); TT(e_, dj[4], dj[3], ALU.subtract)
                            TT(vt[:, :, 2, :], e_, c4, ALU.add)
                            a_ = vs(); TT(a_, dj[3], dj[1], ALU.subtract)
                            a2 = vs(); TS(a2, a_, 2.0)
                            TT(vt[:, :, 3, :], b_, a2, ALU.add)
                            TT(vt[:, :, 4, :], b_, a2, ALU.subtract)
                            a4 = vs(); TS(a4, a2, 2.0)
                            t_ = vs(); TT(t_, dj[5], dj[3], ALU.subtract)
                            TT(vt[:, :, 5, :], t_, a4, ALU.subtract)
                            vts[kc] = vt
                        vtiles[c] = vts

                    for j, nr in groups1:
                        for mc in range(2):
                            ps = ps1.tile([128, nr, W], F32, tag="c1ps", name="c1ps")
                            for dr in range(3):
                                nc.tensor.matmul(
                                    ps, w1a_sb[mc, dr],
                                    xa[:, j + dr:j + dr + nr, 0:W],
                                    start=(dr == 0), stop=False)
                            nc.tensor.matmul(ps, w1s_sb[mc],
                                             xa[0:64, j + 2:j + 2 + nr, 2:W + 2],
                                             start=False, stop=False)
                            nc.tensor.matmul(ps, w1r_sb[mc],
                                             xr[:, j:j + nr, 2:W + 2],
                                             start=False, stop=True)
                            nc.scalar.activation(
                                out=q1b[mc][:, j:j + nr, 1:W + 1], in_=ps,
                                func=AF.Relu, bias=b1_sb[mc], scale=1.0)
                        last_slot = j + nr - 1
                        if last_slot in j_to_chunk:
                            emit_vchunk(j_to_chunk[last_slot])

                    ob = obuf.tile([128, rb // 2, 4], F32, tag="ob", name="ob")
                    ob_flat = ob.rearrange("p g f -> p (g f)")
                    for pp in range(rb // 8):      # 8-row pairs of 4-row tiles
                        vts = vtiles[pp]
                        q2t = {}
                        for mc in range(2):
                            mcp = []
                            for q in range(6):
                                mq = ps2.tile([128, 2, W], F32, tag="m", name="m")
                                n_mm = 0
                                for kc in range(2):
                                    for dc in range(3):
                                        nc.tensor.matmul(
                                            mq, w2_sb[q, kc, dc, mc],
                                            vts[kc][:, 0:2, q, dc:dc + W],
                                            start=(n_mm == 0), stop=(n_mm == 5))
                                        n_mm += 1
                                # stage each plane to SBUF fp16 right away so
                                # the PSUM bank frees fast and the inverse
                                # runs at 2x DVE rate
                                mc_ = tupool.tile([128, 2, W], F16, tag="mcp",
                                                  name="mcp", bufs=6)
                                nc.scalar.copy(mc_, mq)
                                mcp.append(mc_)
                            # inverse A^T (F(4,3)):
                            #  out0 = m0+s+t   out1 = d+2u   out2 = s+4t
                            #  out3 = d+8u+m5  (s=m1+m2 d=m1-m2 t=m3+m4 u=m3-m4)
                            TT, TS = nc.vector.tensor_tensor, nc.vector.tensor_scalar_mul

                            def iv():
                                return tupool.tile([128, 2, W], F16, tag="ivs",
                                                   name="ivs", bufs=9)
                            s_ = iv(); TT(s_, mcp[1], mcp[2], ALU.add)
                            d_ = iv(); TT(d_, mcp[1], mcp[2], ALU.subtract)
                            t_ = iv(); TT(t_, mcp[3], mcp[4], ALU.add)
                            u_ = iv(); TT(u_, mcp[3], mcp[4], ALU.subtract)
                            q2p = q2prep.tile([128, 2, 4, W], F16,
                                              tag=f"q2p{mc}", name=f"q2p{mc}")
                            o0 = iv(); TT(o0, mcp[0], s_, ALU.add)
                            TT(q2p[:, :, 0, :], o0, t_, ALU.add)
                            u2 = iv(); TS(u2, u_, 2.0)
                            TT(q2p[:, :, 1, :], d_, u2, ALU.add)
                            t4 = iv(); TS(t4, t_, 4.0)
                            TT(q2p[:, :, 2, :], s_, t4, ALU.add)
                            u8 = iv(); TS(u8, u2, 4.0)
                            o3 = iv(); TT(o3, d_, u8, ALU.add)
                            TT(q2p[:, :, 3, :], o3, mcp[5], ALU.add)
                            q2t[mc] = q2pool.tile([128, 8, W], F16,
                                                  tag=f"q2_{mc}", name=f"q2_{mc}")
                            nc.scalar.activation(
                                out=q2t[mc], in_=q2p.rearrange("p a b c -> p (a b) c"),
                                func=AF.Relu, bias=b2_sb[mc], scale=1.0)
                        # register this pair's logits + softmax for
                        # deferred emission
                        flush_pending()
                        pl = psl.tile([128, 16, K], F32, tag="lps", name="pl")
                        qf = {kc: q2t[kc].rearrange("p a b -> p (a b)")
                              for kc in range(2)}

                        def lmm(pl=pl, qf=qf):
                            for jj in range(16):
                                for kc in range(2):
                                    yield lambda jj=jj, kc=kc: nc.tensor.matmul(
                                        pl[:, jj, :],
                                        qf[kc][:, 128 * jj:128 * (jj + 1)],
                                        muw_sb[kc],
                                        start=(kc == 0), stop=(kc == 1))
                        pend['mms'] = list(lmm())

                        def fin(pl=pl, ob_flat=ob_flat, pp=pp, r0=r0, rb=rb):
                            # ACT stages pl out of PSUM so the bank frees
                            # without waiting on the (deep) DVE queue
                            pls = smx.tile([128, 16, K], F32, tag="pls", name="pls")
                            nc.scalar.copy(pls, pl)
                            # softmax over K (free axis) + label contraction
                            lg = smx.tile([128, 16, K], F32, tag="li", name="lg")
                            nc.vector.tensor_tensor(
                                lg, pls,
                                bp_sb.unsqueeze(1).to_broadcast([128, 16, K]),
                                ALU.add)
                            mx = smx.tile([128, 16], F32, tag="mx", name="mx")
                            nc.vector.reduce_max(mx, lg, axis=AX.X)
                            ls = smx.tile([128, 16, K], F32, tag="ls", name="ls")
                            nc.vector.tensor_tensor(
                                ls, lg,
                                mx.unsqueeze(2).to_broadcast([128, 16, K]),
                                ALU.subtract)
                            ex = smx.tile([128, 16, K], F32, tag="ex", name="ex")
                            nc.scalar.activation(out=ex, in_=ls, func=AF.Exp)
                            el = smx.tile([128, 16, K], F32, tag="el", name="el")
                            nc.vector.tensor_tensor(
                                el, ex,
                                lab_sb.unsqueeze(1).to_broadcast([128, 16, K]),
                                ALU.mult)
                            ssum = smx.tile([128, 16], F32, tag="ssum", name="ssum")
                            nc.vector.reduce_sum(ssum, ex, axis=AX.X)
                            wsum = smx.tile([128, 16], F32, tag="wsum", name="wsum")
                            nc.vector.reduce_sum(wsum, el, axis=AX.X)
                            rs = smx.tile([128, 16], F32, tag="rs", name="rs")
                            nc.vector.reciprocal(rs, ssum)
                            nc.vector.tensor_tensor(
                                ob_flat[:, 16 * pp:16 * pp + 16], wsum, rs, ALU.mult)
                            if pp == rb // 8 - 1:
                                # contiguous store; host unscrambles
                                # col = 2*row + jj
                                nc.sync.dma_start(
                                    out=outd.ap()[:, r0 * 2:(r0 + rb) * 2],
                                    in_=ob_flat)
                        pend['fin'] = fin
                flush_pending()

    nc.compile()
    return nc


def prep_inputs(x, w1, b1, w2, b2, w3, b3, mu, label):
    """Full inputs -> per-core in_maps."""
    w3m = w3[:, :, 0, 0]
    muW = 2.0 * (mu @ w3m)                                   # [K, Q]
    bpv = (2.0 * (mu @ b3) - (mu * mu).sum(1)).astype(np.float32)

    def pack_w(w1f, w2f):
        w1a = np.empty((2, 3, 128, 128), np.float32)
        w1r = np.empty((2, 128, 128), np.float32)
        w1s = np.empty((2, 64, 128), np.float32)
        for mc in range(2):
            ms = slice(128 * mc, 128 * (mc + 1))
            for dr in range(3):
                w1a[mc, dr, 0:64] = w1f[ms, :, dr, 0].T
                w1a[mc, dr, 64:128] = w1f[ms, :, dr, 1].T
            w1r[mc, 0:64] = w1f[ms, :, 0, 2].T
            w1r[mc, 64:128] = w1f[ms, :, 1, 2].T
            w1s[mc] = w1f[ms, :, 2, 2].T
        # Winograd F(4,3) along H for conv2: Gw[p] = sum_dr G[p,dr] w2[..dr..]
        G = np.array([[1 / 4, 0, 0],
                      [-1 / 6, -1 / 6, -1 / 6],
                      [-1 / 6, 1 / 6, -1 / 6],
                      [1 / 24, 1 / 12, 1 / 6],
                      [1 / 24, -1 / 12, 1 / 6],
                      [0, 0, 1]], np.float32)
        gw = np.einsum('pd,ocdk->pock', G, w2f.astype(np.float32))
        w2p = np.empty((72, 128, 128), np.float32)
        for p in range(6):
            for kc in range(2):
                for dc in range(3):
                    for mc in range(2):
                        idx = ((p * 2 + kc) * 3 + dc) * 2 + mc
                        w2p[idx] = gw[p, 128 * mc:128 * (mc + 1),
                                      128 * kc:128 * (kc + 1), dc].T
        w1xp = np.concatenate([w1a.reshape(6, 128, 128), w1r], axis=0)
        return (np.ascontiguousarray(w1xp.transpose(1, 0, 2)).astype(np.float16),
                np.ascontiguousarray(w1s.transpose(1, 0, 2)).astype(np.float16),
                np.ascontiguousarray(w2p.transpose(1, 0, 2)).astype(np.float16))

    packs = {}
    packs[0] = pack_w(w1, w2)
    packs[1] = pack_w(w1[:, :, ::-1, :], w2[:, :, ::-1, :])

    muwp = np.empty((128, 2, K), np.float32)
    for kc in range(2):
        muwp[:, kc, :] = muW[:, 128 * kc:128 * (kc + 1)].T
    muwp = muwp.astype(np.float16)
    cstv = np.empty((128, 2 * K + 4), np.float32)
    cstv[:, 0:K] = bpv[None, :]
    cstv[:, K:2 * K] = label[None, :].astype(np.float32)
    for mc in range(2):
        cstv[:, 2 * K + mc] = b1[128 * mc:128 * (mc + 1)]
        cstv[:, 2 * K + 2 + mc] = b2[128 * mc:128 * (mc + 1)]

    rows = np.clip(np.arange(132) - 2, 0, H - 1)
    cols = np.clip(np.arange(W + 2) - 1, 0, W - 1)
    in_maps = []
    for core in range(NCORES):
        img, half = core // 2, core % 2
        xl = x[img] if half == 0 else x[img, :, ::-1, :]
        xhv = np.ascontiguousarray(xl[:, rows][:, :, cols]).astype(np.float16)
        xhf = np.concatenate([xhv.reshape(-1),
                              np.zeros(2 * (W + 2), np.float16)])
        w1xp, w1sp, w2p = packs[half]
        in_maps.append({
            'xh': xhf, 'w1x': w1xp, 'w1s': w1sp, 'w2l': w2p,
            'muw': muwp, 'cst': cstv,
        })
    return in_maps


def gather(results, dtype=np.float32):
    out = np.empty((B, 1, H, W), dtype)
    for core in range(NCORES):
        img, half = core // 2, core % 2
        o = results[core]['out']        # [128 p, 2*row + jj]
        o = o.reshape(128, 128, 2).transpose(1, 2, 0).reshape(128, W)
        if half == 0:
            out[img, 0, 0:128] = o
        else:
            out[img, 0, 128:256] = o[::-1]
    return out


def get_nc():
    if 'nc' not in _cached:
        _cached['nc'] = build_nc()
    return _cached['nc']


def kernel(x, w1, b1, w2, b2, w3, b3, mu, label, **run_kwargs):
    nc = get_nc()
    in_maps = prep_inputs(
        np.asarray(x, np.float32), np.asarray(w1, np.float32),
        np.asarray(b1, np.float32), np.asarray(w2, np.float32),
        np.asarray(b2, np.float32), np.asarray(w3, np.float32),
        np.asarray(b3, np.float32), np.asarray(mu, np.float32),
        np.asarray(label, np.float32))
    res = run_bass_kernel_spmd(nc, in_maps, core_ids=list(range(NCORES)),
                               **run_kwargs)
    out = gather(res.results)
    if run_kwargs:
        _cached['last_result'] = res
    return out


# revision 35
# speedup vs baseline: 1.1893x; 1.1893x over previous
"""AttentionClustering (vq_codebook) Trainium2 kernel, 8-core data parallel.

Shard: 8 cores = 4 images x 2 half-images (128 output rows each). Odd cores
get a vertically flipped shard + row-flipped conv weights so every core's
program is identical (true image edge at local top, interior halo at bottom).

Math: q1 = relu(conv3x3(x, w1) + b1); q2 = relu(conv3x3(q1, w2) + b2)  (both
with replicate padding); then the 1x1 conv + cluster-distance softmax folds to
  logit[px, k] = sum_ci q2[ci, px] * muW[k, ci] + bp[k]
  muW = 2 * mu @ W3,  bp = 2 * mu @ b3 - |mu|^2      (|q|^2 cancels in softmax)
  out[px] = sum_k softmax_k(logit) * label[k]

conv2 (the FLOP-dominant layer) runs as Winograd F(2,3) along H: for each
output row pair, V_p = B^T [q1 rows 2i-1..2i+2] (4 DVE adds per 4 tile-rows),
m_p = sum_{kc,dc} Gw2[p,kc,dc] @ V_p[:, dc:dc+256] (24 matmuls of N=512 per
i-PAIR per mc vs 36 for direct conv -> 1.5x fewer PE columns), then
out_even = m0+m1+m2, out_odd = m1-m2-m3 (4 DVE adds) + ACT relu. conv1 stays
direct (dual-tap packed, near-ideal).

All matmuls run in fp16: full PE rate at any N, fp16 LDWEIGHTS hides under the
N=512 streams. fp32 accumulate in PSUM.
"""
import sys
if '/opt/trn_rl_repo' not in sys.path:
    sys.path.insert(0, '/opt/trn_rl_repo')

import numpy as np
import concourse.bass as bass
import concourse.mybir as mybir
from concourse import bacc, tile
from concourse.bass_utils import run_bass_kernel_spmd

F32 = mybir.dt.float32
F16 = mybir.dt.float16
F8 = mybir.dt.float8e3
AF = mybir.ActivationFunctionType
ALU = mybir.AluOpType
AX = mybir.AxisListType

B, CIN, H, W = 4, 64, 256, 256
Q, K = 256, 16
RB = 32           # output rows per band
NBAND = 4         # bands per core (128 rows)
NCORES = 8

_cached = {}


def build_nc():
    nc = bacc.Bacc("TRN2", target_bir_lowering=False, debug=False)

    CHS = 132 * (W + 2)          # per-channel element stride in flat xh
    xh = nc.declare_dram_parameter("xh", [CIN * CHS + 2 * (W + 2)], F16,
                                   isOutput=False)
    w1x = nc.declare_dram_parameter("w1x", [128, 8, 128], F16, isOutput=False)
    w1s = nc.declare_dram_parameter("w1s", [64, 2, 128], F16, isOutput=False)
    w2l = nc.declare_dram_parameter("w2l", [128, 72, 128], F16, isOutput=False)
    muw = nc.declare_dram_parameter("muw", [128, 2, K], F16, isOutput=False)
    cst = nc.declare_dram_parameter("cst", [128, 2 * K + 4], F32, isOutput=False)
    # out stays in the on-chip [px-partition, band, g, r, jj] layout; the host
    # unscrambles. A row-major dram layout would need 4 B DMA descriptors.
    outd = nc.declare_dram_parameter("out", [128, NBAND * RB * 2], F32,
                                     isOutput=True)

    with tile.TileContext(nc) as tc:
        with tc.tile_pool(name="singles", bufs=1) as singles, \
             tc.tile_pool(name="xpool", bufs=1) as xpool, \
             tc.tile_pool(name="q1pool", bufs=1) as q1pool, \
             tc.tile_pool(name="vpool", bufs=2) as vpool, \
             tc.tile_pool(name="vscr", bufs=8) as vscr, \
             tc.tile_pool(name="tupool", bufs=2) as tupool, \
             tc.tile_pool(name="q2pre", bufs=1) as q2prep, \
             tc.tile_pool(name="q2pool", bufs=2) as q2pool, \
             tc.tile_pool(name="smx", bufs=1) as smx, \
             tc.tile_pool(name="obuf", bufs=2) as obuf:

            # ---- resident weights (one DMA per family) ----------------
            w1xbuf = singles.tile([128, 8, 128], F16, tag="w1xbuf")
            nc.sync.dma_start(out=w1xbuf, in_=w1x.ap())
            w1a_sb = {(mc, dr): w1xbuf[:, mc * 3 + dr, :]
                      for mc in range(2) for dr in range(3)}
            w1r_sb = {mc: w1xbuf[:, 6 + mc, :] for mc in range(2)}
            w1sbuf = singles.tile([64, 2, 128], F16, tag="w1sbuf")
            nc.sync.dma_start(out=w1sbuf, in_=w1s.ap())
            w1s_sb = {mc: w1sbuf[:, mc, :] for mc in range(2)}
            # band-0 x halo first so conv1 can start before w2 finishes loading
            xh_ap = xh.ap()

            def xsrc(r0, lo, hi, shift):
                # [64ch, elems] slice of the flat xh, shifted by `shift`
                # elements (1 = one column, W+2 = one row). Rows are contiguous
                # in dram, so one big run per channel keeps the DMA
                # descriptor count at 64/transfer instead of 64*rows.
                return bass.AP(
                    tensor=xh_ap.tensor,
                    offset=(r0 + lo) * (W + 2) + shift,
                    ap=[[CHS, CIN], [1, (hi - lo) * (W + 2)]])

            def load_xband(r0, rb):
                chunks = [(0, rb + 4)]
                xa = xpool.tile([128, rb + 4, W + 2], F16, tag="xa", name="xa", bufs=2)
                xr = xpool.tile([128, rb + 4, W + 2], F16, tag="xr", name="xr")
                xaf = xa.rearrange("p r c -> p (r c)")
                xrf = xr.rearrange("p r c -> p (r c)")
                for lo, hi in chunks:
                    cl, ch = lo * (W + 2), hi * (W + 2)
                    nc.sync.dma_start(out=xaf[0:64, cl:ch],
                                      in_=xsrc(r0, lo, hi, 0))
                    nc.sync.dma_start(out=xaf[64:128, cl:ch],
                                      in_=xsrc(r0, lo, hi, 1))
                    nc.sync.dma_start(out=xrf[0:64, cl:ch],
                                      in_=xsrc(r0, lo, hi, 0))
                    nc.sync.dma_start(out=xrf[64:128, cl:ch],
                                      in_=xsrc(r0, lo, hi, W + 2))
                return xa, xr

            xband0 = load_xband(0, 16)

            # small constants next (before the bulky w2 tiles hog the queues)
            muwbuf = singles.tile([128, 2, K], F16, tag="muwbuf")
            nc.sync.dma_start(out=muwbuf, in_=muw.ap())
            muw_sb = {kc: muwbuf[:, kc, :] for kc in range(2)}
            cstbuf = singles.tile([128, 2 * K + 4], F32, tag="cstbuf")
            nc.sync.dma_start(out=cstbuf, in_=cst.ap())
            bp_sb = cstbuf[:, 0:K]
            lab_sb = cstbuf[:, K:2 * K]
            cb16 = singles.tile([128, 4], F16, tag="cb16")
            nc.vector.tensor_copy(cb16, cstbuf[:, 2 * K:2 * K + 4])
            b1_sb = {mc: cb16[:, mc:mc + 1] for mc in range(2)}
            b2_sb = {mc: cb16[:, 2 + mc:3 + mc] for mc in range(2)}

            # PE warmup: keep TensorE busy through the initial DMA wait so
            # the HAM clock-gate is at 8/8 when real matmuls arrive.
            wscr = singles.tile([128, 512], F16, tag="wscr")
            nc.vector.memset(wscr, 0.0)
            with tc.tile_pool(name="psw", bufs=1, space="PSUM") as psw:
                wps = psw.tile([128, 512], F32, tag="wps", name="wps")
                for _ in range(46):
                    nc.tensor.matmul(wps, wscr[:, 0:128], wscr,
                                     start=True, stop=True)

            w2buf = singles.tile([128, 72, 128], F16, tag="w2buf")
            nc.sync.dma_start(out=w2buf[:, 0:24, :], in_=w2l.ap()[:, 0:24, :])
            nc.sync.dma_start(out=w2buf[:, 24:72, :], in_=w2l.ap()[:, 24:72, :])
            # Winograd-transformed conv2 weights: index (pos, kc, dc, mc)
            w2_sb = {(p, kc, dc, mc): w2buf[:, ((p * 2 + kc) * 3 + dc) * 2 + mc, :]
                     for p in range(6) for kc in range(2)
                     for dc in range(3) for mc in range(2)}

            with tc.tile_pool(name="ps1", bufs=2, space="PSUM") as ps1, \
                 tc.tile_pool(name="ps2", bufs=5, space="PSUM") as ps2, \
                 tc.tile_pool(name="psl", bufs=1, space="PSUM") as psl:

                # Logits matmuls of ipair p are deferred and interleaved
                # into the next ipair's conv2 stream: their ~97ns LDWEIGHTS
                # then hides under the N=512 conv2 streams (the LDW port has
                # 216-2*97 ns of slack per conv2 MM) instead of serializing.
                pend = {'mms': [], 'fin': None}

                def emit_pending_mms(k):
                    while k > 0 and pend['mms']:
                        pend['mms'].pop(0)()
                        k -= 1

                def flush_pending():
                    while pend['mms']:
                        pend['mms'].pop(0)()
                    if pend['fin'] is not None:
                        pend['fin']()
                        pend['fin'] = None

                # ---- bands: band 0 split in half so the first x halo
                # load (and hence conv1) is ready sooner ------------------
                BANDS = [(0, 16), (16, 16), (32, 32), (64, 32), (96, 32)]
                for bi, (r0, rb) in enumerate(BANDS):
                    # x halo in two packings:
                    #  xa: p0-63 = xh rows, p64-127 = same shifted +1 col
                    #  xr: p0-63 = xh rows, p64-127 = same shifted +1 row
                    xa, xr = xband0 if bi == 0 else load_xband(r0, rb)

                    # q1 band buffer: slot j = q1 row (r0 - 1 + j), cols
                    # 1..256 real, cols 0/257 replicate pads.
                    q1b = {}
                    nslot = ((rb + 2 + 3) // 4) * 4
                    for kc in range(2):
                        q1b[kc] = q1pool.tile([128, nslot, W + 2], F16,
                                              tag=f"q1_{kc}", name=f"q1_{kc}")
                    q1v = {kc: q1b[kc].rearrange("p (s four) c -> p s four c", four=4)
                           for kc in range(2)}

                    # conv1: q1 slot j needs xh local rows j+dr (pairs), and
                    # taps (0,2),(1,2) from xr row j, tap (2,2) from xa row j+2.
                    nch = rb // 8
                    if bi == 0:
                        groups1 = [(j, 2) for j in range(1, rb + 1, 2)] + [(rb + 1, 1)]
                        ready_j = {c: 8 * c + 10 for c in range(nch - 1)}
                        ready_j[nch - 1] = rb + 1
                    else:
                        groups1 = [(j, 2) for j in range(0, rb + 2, 2)]
                        ready_j = {c: 8 * c + 9 for c in range(nch)}
                    j_to_chunk = {v: k for k, v in ready_j.items()}

                    vtiles = {}

                    def emit_vchunk(c):
                        # replicate-pad cols for the slot range this chunk reads
                        lo = (1 if bi == 0 else 0) if c == 0 else 8 * c + 2
                        hi = 8 * c + 10          # exclusive
                        for kc in range(2):
                            nc.vector.tensor_copy(
                                out=q1b[kc][:, lo:hi, 0:1],
                                in_=q1b[kc][:, lo:hi, 1:2])
                            nc.vector.tensor_copy(
                                out=q1b[kc][:, lo:hi, W + 1:W + 2],
                                in_=q1b[kc][:, lo:hi, W:W + 1])
                        if bi == 0 and c == 0:
                            for kc in range(2):
                                nc.vector.tensor_copy(
                                    out=q1b[kc][:, 0:1, :], in_=q1b[kc][:, 1:2, :])
                        # B^T row-transform (F(4,3)): 6 V planes for the two
                        # 4-row tiles i4 in {2c, 2c+1}; d_j = slot 4*i4 + j.
                        #  v0 = (d4-d2) - 4(d2-d0)      v1 = (d3+d4) - 4(d1+d2)
                        #  v2 = (d4-d3) + 4(d1-d2)      v3 = (d4-d2) + 2(d3-d1)
                        #  v4 = (d4-d2) - 2(d3-d1)      v5 = (d5-d3) - 4(d3-d1)
                        vts = {}
                        for kc in range(2):
                            dj = [q1v[kc][:, 2 * c + (j // 4):2 * c + (j // 4) + 2,
                                          j % 4, :] for j in range(6)]
                            vt = vpool.tile([128, 2, 6, W + 2], F16,
                                            tag=f"v{kc}", name=f"v{kc}")

                            def vs():
                                return vscr.tile([128, 2, W + 2], F16,
                                                 tag="vs", name="vs", bufs=8)
                            TT, TS = nc.vector.tensor_tensor, nc.vector.tensor_scalar_mul
                            # planes emitted in conv2 consumption order (v0..v5)
                            b_ = vs(); TT(b_, dj[4], dj[2], ALU.subtract)
                            h_ = vs(); TT(h_, dj[2], dj[0], ALU.subtract)
                            h4 = vs(); TS(h4, h_, 4.0)
                            TT(vt[:, :, 0, :], b_, h4, ALU.subtract)
                            f_ = vs(); TT(f_, dj[1], dj[2], ALU.add)
                            f4 = vs(); TS(f4, f_, 4.0)
                            g_ = vs(); TT(g_, dj[3], dj[4], ALU.add)
                            TT(vt[:, :, 1, :], g_, f4, ALU.subtract)
                            c_ = vs(); TT(c_, dj[1], dj[2], ALU.subtract)
                            c4 = vs(); TS(c4, c_, 4.0)
                            e_ = vs(); TT(e_, dj[4], dj[3], ALU.subtract)
                            TT(vt[:, :, 2, :], e_, c4, ALU.add)
                            a_ = vs(); TT(a_, dj[3], dj[1], ALU.subtract)
                            a2 = vs(); TS(a2, a_, 2.0)
                            TT(vt[:, :, 3, :], b_, a2, ALU.add)
                            TT(vt[:, :, 4, :], b_, a2, ALU.subtract)
                            a4 = vs(); TS(a4, a2, 2.0)
                            t_ = vs(); TT(t_, dj[5], dj[3], ALU.subtract)
                            TT(vt[:, :, 5, :], t_, a4, ALU.subtract)
                            vts[kc] = vt
                        vtiles[c] = vts

                    for j, nr in groups1:
                        for mc in range(2):
                            ps = ps1.tile([128, nr, W], F32, tag="c1ps", name="c1ps")
                            for dr in range(3):
                                nc.tensor.matmul(
                                    ps, w1a_sb[mc, dr],
                                    xa[:, j + dr:j + dr + nr, 0:W],
                                    start=(dr == 0), stop=False)
                            nc.tensor.matmul(ps, w1s_sb[mc],
                                             xa[0:64, j + 2:j + 2 + nr, 2:W + 2],
                                             start=False, stop=False)
                            nc.tensor.matmul(ps, w1r_sb[mc],
                                             xr[:, j:j + nr, 2:W + 2],
                                             start=False, stop=True)
                            nc.scalar.activation(
                                out=q1b[mc][:, j:j + nr, 1:W + 1], in_=ps,
                                func=AF.Relu, bias=b1_sb[mc], scale=1.0)
                        last_slot = j + nr - 1
                        if last_slot in j_to_chunk:
                            emit_vchunk(j_to_chunk[last_slot])

                    ob = obuf.tile([128, rb // 2, 4], F32, tag="ob", name="ob")
                    ob_flat = ob.rearrange("p g f -> p (g f)")
                    for pp in range(rb // 8):      # 8-row pairs of 4-row tiles
                        vts = vtiles[pp]
                        q2t = {}
                        for mc in range(2):
                            mcp = []
                            for q in range(6):
                                mq = ps2.tile([128, 2, W], F32, tag="m", name="m")
                                n_mm = 0
                                for kc in range(2):
                                    for dc in range(3):
                                        nc.tensor.matmul(
                                            mq, w2_sb[q, kc, dc, mc],
                                            vts[kc][:, 0:2, q, dc:dc + W],
                                            start=(n_mm == 0), stop=(n_mm == 5))
                                        n_mm += 1
                                # stage each plane to SBUF fp16 right away so
                                # the PSUM bank frees fast and the inverse
                                # runs at 2x DVE rate
                                mc_ = tupool.tile([128, 2, W], F16, tag="mcp",
                                                  name="mcp", bufs=6)
                                nc.scalar.copy(mc_, mq)
                                mcp.append(mc_)
                            # inverse A^T (F(4,3)):
                            #  out0 = m0+s+t   out1 = d+2u   out2 = s+4t
                            #  out3 = d+8u+m5  (s=m1+m2 d=m1-m2 t=m3+m4 u=m3-m4)
                            TT, TS = nc.vector.tensor_tensor, nc.vector.tensor_scalar_mul

                            def iv():
                                return tupool.tile([128, 2, W], F16, tag="ivs",
                                                   name="ivs", bufs=9)
                            s_ = iv(); TT(s_, mcp[1], mcp[2], ALU.add)
                            d_ = iv(); TT(d_, mcp[1], mcp[2], ALU.subtract)
                            t_ = iv(); TT(t_, mcp[3], mcp[4], ALU.add)
                            u_ = iv(); TT(u_, mcp[3], mcp[4], ALU.subtract)
                            q2p = q2prep.tile([128, 2, 4, W], F16,
                                              tag=f"q2p{mc}", name=f"q2p{mc}")
                            o0 = iv(); TT(o0, mcp[0], s_, ALU.add)
                            TT(q2p[:, :, 0, :], o0, t_, ALU.add)
                            u2 = iv(); TS(u2, u_, 2.0)
                            TT(q2p[:, :, 1, :], d_, u2, ALU.add)
                            t4 = iv(); TS(t4, t_, 4.0)
                            TT(q2p[:, :, 2, :], s_, t4, ALU.add)
                            u8 = iv(); TS(u8, u2, 4.0)
                            o3 = iv(); TT(o3, d_, u8, ALU.add)
                            TT(q2p[:, :, 3, :], o3, mcp[5], ALU.add)
                            q2t[mc] = q2pool.tile([128, 8, W], F16,
                                                  tag=f"q2_{mc}", name=f"q2_{mc}")
                            nc.scalar.activation(
                                out=q2t[mc], in_=q2p.rearrange("p a b c -> p (a b) c"),
                                func=AF.Relu, bias=b2_sb[mc], scale=1.0)
                        # register this pair's logits + softmax for
                        # deferred emission
                        flush_pending()
                        pl = psl.tile([128, 16, K], F32, tag="lps", name="pl")
                        qf = {kc: q2t[kc].rearrange("p a b -> p (a b)")
                              for kc in range(2)}

                        def lmm(pl=pl, qf=qf):
                            for jj in range(16):
                                for kc in range(2):
                                    yield lambda jj=jj, kc=kc: nc.tensor.matmul(
                                        pl[:, jj, :],
                                        qf[kc][:, 128 * jj:128 * (jj + 1)],
                                        muw_sb[kc],
                                        start=(kc == 0), stop=(kc == 1))
                        pend['mms'] = list(lmm())

                        def fin(pl=pl, ob_flat=ob_flat, pp=pp, r0=r0, rb=rb):
                            # ACT stages pl out of PSUM so the bank frees
                            # without waiting on the (deep) DVE queue
                            pls = smx.tile([128, 16, K], F32, tag="pls", name="pls")
                            nc.scalar.copy(pls, pl)
                            # softmax over K (free axis) + label contraction
                            lg = smx.tile([128, 16, K], F32, tag="li", name="lg")
                            nc.vector.tensor_tensor(
                                lg, pls,
                                bp_sb.unsqueeze(1).to_broadcast([128, 16, K]),
                                ALU.add)
                            mx = smx.tile([128, 16], F32, tag="mx", name="mx")
                            nc.vector.reduce_max(mx, lg, axis=AX.X)
                            ls = smx.tile([128, 16, K], F32, tag="ls", name="ls")
                            nc.vector.tensor_tensor(
                                ls, lg,
                                mx.unsqueeze(2).to_broadcast([128, 16, K]),
                                ALU.subtract)
                            ex = smx.tile([128, 16, K], F32, tag="ex", name="ex")
                            nc.scalar.activation(out=ex, in_=ls, func=AF.Exp)
                            el = smx.tile([128, 16, K], F32, tag="el", name="el")
                            nc.vector.tensor_tensor(
                                el, ex,
                                lab_sb.unsqueeze(1).to_broadcast([128, 16, K]),
                                ALU.mult)
                            ssum = smx.tile([128, 16], F32, tag="ssum", name="ssum")
                            nc.vector.reduce_sum(ssum, ex, axis=AX.X)
                            wsum = smx.tile([128, 16], F32, tag="wsum", name="wsum")
                            nc.vector.reduce_sum(wsum, el, axis=AX.X)
                            rs = smx.tile([128, 16], F32, tag="rs", name="rs")
                            nc.vector.reciprocal(rs, ssum)
                            nc.vector.tensor_tensor(
                                ob_flat[:, 16 * pp:16 * pp + 16], wsum, rs, ALU.mult)
                            if pp == rb // 8 - 1:
                                # contiguous store; host unscrambles
                                # col = 2*row + jj
                                nc.sync.dma_start(
                                    out=outd.ap()[:, r0 * 2:(r0 + rb) * 2],
                                    in_=ob_flat)
                        pend['fin'] = fin
                flush_pending()

    nc.compile()
    return nc


def prep_inputs(x, w1, b1, w2, b2, w3, b3, mu, label):
    """Full inputs -> per-core in_maps."""
    w3m = w3[:, :, 0, 0]
    muW = 2.0 * (mu @ w3m)                                   # [K, Q]
    bpv = (2.0 * (mu @ b3) - (mu * mu).sum(1)).astype(np.float32)

    def pack_w(w1f, w2f):
        w1a = np.empty((2, 3, 128, 128), np.float32)
        w1r = np.empty((2, 128, 128), np.float32)
        w1s = np.empty((2, 64, 128), np.float32)
        for mc in range(2):
            ms = slice(128 * mc, 128 * (mc + 1))
            for dr in range(3):
                w1a[mc, dr, 0:64] = w1f[ms, :, dr, 0].T
                w1a[mc, dr, 64:128] = w1f[ms, :, dr, 1].T
            w1r[mc, 0:64] = w1f[ms, :, 0, 2].T
            w1r[mc, 64:128] = w1f[ms, :, 1, 2].T
            w1s[mc] = w1f[ms, :, 2, 2].T
        # Winograd F(4,3) along H for conv2: Gw[p] = sum_dr G[p,dr] w2[..dr..]
        G = np.array([[1 / 4, 0, 0],
                      [-1 / 6, -1 / 6, -1 / 6],
                      [-1 / 6, 1 / 6, -1 / 6],
                      [1 / 24, 1 / 12, 1 / 6],
                      [1 / 24, -1 / 12, 1 / 6],
                      [0, 0, 1]], np.float32)
        gw = np.einsum('pd,ocdk->pock', G, w2f.astype(np.float32))
        w2p = np.empty((72, 128, 128), np.float32)
        for p in range(6):
            for kc in range(2):
                for dc in range(3):
                    for mc in range(2):
                        idx = ((p * 2 + kc) * 3 + dc) * 2 + mc
                        w2p[idx] = gw[p, 128 * mc:128 * (mc + 1),
                                      128 * kc:128 * (kc + 1), dc].T
        w1xp = np.concatenate([w1a.reshape(6, 128, 128), w1r], axis=0)
        return (np.ascontiguousarray(w1xp.transpose(1, 0, 2)).astype(np.float16),
                np.ascontiguousarray(w1s.transpose(1, 0, 2)).astype(np.float16),
                np.ascontiguousarray(w2p.transpose(1, 0, 2)).astype(np.float16))

    packs = {}
    packs[0] = pack_w(w1, w2)
    packs[1] = pack_w(w1[:, :, ::-1, :], w2[:, :, ::-1, :])

    muwp = np.empty((128, 2, K), np.float32)
    for kc in range(2):
        muwp[:, kc, :] = muW[:, 128 * kc:128 * (kc + 1)].T
    muwp = muwp.astype(np.float16)
    cstv = np.empty((128, 2 * K + 4), np.float32)
    cstv[:, 0:K] = bpv[None, :]
    cstv[:, K:2 * K] = label[None, :].astype(np.float32)
    for mc in range(2):
        cstv[:, 2 * K + mc] = b1[128 * mc:128 * (mc + 1)]
        cstv[:, 2 * K + 2 + mc] = b2[128 * mc:128 * (mc + 1)]

    rows = np.clip(np.arange(132) - 2, 0, H - 1)
    cols = np.clip(np.arange(W + 2) - 1, 0, W - 1)
    in_maps = []
    for core in range(NCORES):
        img, half = core // 2, core % 2
        xl = x[img] if half == 0 else x[img, :, ::-1, :]
        xhv = np.ascontiguousarray(xl[:, rows][:, :, cols]).astype(np.float16)
        xhf = np.concatenate([xhv.reshape(-1),
                              np.zeros(2 * (W + 2), np.float16)])
        w1xp, w1sp, w2p = packs[half]
        in_maps.append({
            'xh': xhf, 'w1x': w1xp, 'w1s': w1sp, 'w2l': w2p,
            'muw': muwp, 'cst': cstv,
        })
    return in_maps


def gather(results, dtype=np.float32):
    out = np.empty((B, 1, H, W), dtype)
    for core in range(NCORES):
        img, half = core // 2, core % 2
        o = results[core]['out']        # [128 p, 2*row + jj]
        o = o.reshape(128, 128, 2).transpose(1, 2, 0).reshape(128, W)
        if half == 0:
            out[img, 0, 0:128] = o
        else:
            out[img, 0, 128:256] = o[::-1]
    return out


def get_nc():
    if 'nc' not in _cached:
        _cached['nc'] = build_nc()
    return _cached['nc']


def kernel(x, w1, b1, w2, b2, w3, b3, mu, label, **run_kwargs):
    nc = get_nc()
    in_maps = prep_inputs(
        np.asarray(x, np.float32), np.asarray(w1, np.float32),
        np.asarray(b1, np.float32), np.asarray(w2, np.float32),
        np.asarray(b2, np.float32), np.asarray(w3, np.float32),
        np.asarray(b3, np.float32), np.asarray(mu, np.float32),
        np.asarray(label, np.float32))
    res = run_bass_kernel_spmd(nc, in_maps, core_ids=list(range(NCORES)),
                               **run_kwargs)
    out = gather(res.results)
    if run_kwargs:
        _cached['last_result'] = res
    return out


# revision 36
# speedup vs baseline: 1.1932x; 1.0033x over previous
"""AttentionClustering (vq_codebook) Trainium2 kernel, 8-core data parallel.

Shard: 8 cores = 4 images x 2 half-images (128 output rows each). Odd cores
get a vertically flipped shard + row-flipped conv weights so every core's
program is identical (true image edge at local top, interior halo at bottom).

Math: q1 = relu(conv3x3(x, w1) + b1); q2 = relu(conv3x3(q1, w2) + b2)  (both
with replicate padding); then the 1x1 conv + cluster-distance softmax folds to
  logit[px, k] = sum_ci q2[ci, px] * muW[k, ci] + bp[k]
  muW = 2 * mu @ W3,  bp = 2 * mu @ b3 - |mu|^2      (|q|^2 cancels in softmax)
  out[px] = sum_k softmax_k(logit) * label[k]

conv2 (the FLOP-dominant layer) runs as Winograd F(2,3) along H: for each
output row pair, V_p = B^T [q1 rows 2i-1..2i+2] (4 DVE adds per 4 tile-rows),
m_p = sum_{kc,dc} Gw2[p,kc,dc] @ V_p[:, dc:dc+256] (24 matmuls of N=512 per
i-PAIR per mc vs 36 for direct conv -> 1.5x fewer PE columns), then
out_even = m0+m1+m2, out_odd = m1-m2-m3 (4 DVE adds) + ACT relu. conv1 stays
direct (dual-tap packed, near-ideal).

All matmuls run in fp16: full PE rate at any N, fp16 LDWEIGHTS hides under the
N=512 streams. fp32 accumulate in PSUM.
"""
import sys
if '/opt/trn_rl_repo' not in sys.path:
    sys.path.insert(0, '/opt/trn_rl_repo')

import numpy as np
import concourse.bass as bass
import concourse.mybir as mybir
from concourse import bacc, tile
from concourse.bass_utils import run_bass_kernel_spmd

F32 = mybir.dt.float32
F16 = mybir.dt.float16
F8 = mybir.dt.float8e3
AF = mybir.ActivationFunctionType
ALU = mybir.AluOpType
AX = mybir.AxisListType

B, CIN, H, W = 4, 64, 256, 256
Q, K = 256, 16
RB = 32           # output rows per band
NBAND = 4         # bands per core (128 rows)
NCORES = 8

_cached = {}


def build_nc():
    nc = bacc.Bacc("TRN2", target_bir_lowering=False, debug=False)

    CHS = 132 * (W + 2)          # per-channel element stride in flat xh
    xh = nc.declare_dram_parameter("xh", [CIN * CHS + 2 * (W + 2)], F16,
                                   isOutput=False)
    w1x = nc.declare_dram_parameter("w1x", [128, 8, 128], F16, isOutput=False)
    w1s = nc.declare_dram_parameter("w1s", [64, 2, 128], F16, isOutput=False)
    w2l = nc.declare_dram_parameter("w2l", [128, 72, 128], F16, isOutput=False)
    muw = nc.declare_dram_parameter("muw", [128, 2, K], F16, isOutput=False)
    cst = nc.declare_dram_parameter("cst", [128, 2 * K + 4], F32, isOutput=False)
    # out stays in the on-chip [px-partition, band, g, r, jj] layout; the host
    # unscrambles. A row-major dram layout would need 4 B DMA descriptors.
    outd = nc.declare_dram_parameter("out", [128, NBAND * RB * 2], F32,
                                     isOutput=True)

    with tile.TileContext(nc) as tc:
        with tc.tile_pool(name="singles", bufs=1) as singles, \
             tc.tile_pool(name="xpool", bufs=1) as xpool, \
             tc.tile_pool(name="q1pool", bufs=1) as q1pool, \
             tc.tile_pool(name="vpool", bufs=2) as vpool, \
             tc.tile_pool(name="vscr", bufs=8) as vscr, \
             tc.tile_pool(name="tupool", bufs=2) as tupool, \
             tc.tile_pool(name="q2pre", bufs=1) as q2prep, \
             tc.tile_pool(name="q2pool", bufs=2) as q2pool, \
             tc.tile_pool(name="smx", bufs=1) as smx, \
             tc.tile_pool(name="obuf", bufs=2) as obuf:

            # ---- resident weights (one DMA per family) ----------------
            w1xbuf = singles.tile([128, 8, 128], F16, tag="w1xbuf")
            nc.sync.dma_start(out=w1xbuf, in_=w1x.ap())
            w1a_sb = {(mc, dr): w1xbuf[:, mc * 3 + dr, :]
                      for mc in range(2) for dr in range(3)}
            w1r_sb = {mc: w1xbuf[:, 6 + mc, :] for mc in range(2)}
            w1sbuf = singles.tile([64, 2, 128], F16, tag="w1sbuf")
            nc.sync.dma_start(out=w1sbuf, in_=w1s.ap())
            w1s_sb = {mc: w1sbuf[:, mc, :] for mc in range(2)}
            # band-0 x halo first so conv1 can start before w2 finishes loading
            xh_ap = xh.ap()

            def xsrc(r0, lo, hi, shift):
                # [64ch, elems] slice of the flat xh, shifted by `shift`
                # elements (1 = one column, W+2 = one row). Rows are contiguous
                # in dram, so one big run per channel keeps the DMA
                # descriptor count at 64/transfer instead of 64*rows.
                return bass.AP(
                    tensor=xh_ap.tensor,
                    offset=(r0 + lo) * (W + 2) + shift,
                    ap=[[CHS, CIN], [1, (hi - lo) * (W + 2)]])

            def load_xband(r0, rb):
                chunks = [(0, rb + 4)]
                xa = xpool.tile([128, rb + 4, W + 2], F16, tag="xa", name="xa", bufs=2)
                xr = xpool.tile([128, rb + 4, W + 2], F16, tag="xr", name="xr")
                xaf = xa.rearrange("p r c -> p (r c)")
                xrf = xr.rearrange("p r c -> p (r c)")
                for lo, hi in chunks:
                    cl, ch = lo * (W + 2), hi * (W + 2)
                    nc.sync.dma_start(out=xaf[0:64, cl:ch],
                                      in_=xsrc(r0, lo, hi, 0))
                    nc.sync.dma_start(out=xaf[64:128, cl:ch],
                                      in_=xsrc(r0, lo, hi, 1))
                    nc.sync.dma_start(out=xrf[0:64, cl:ch],
                                      in_=xsrc(r0, lo, hi, 0))
                    nc.sync.dma_start(out=xrf[64:128, cl:ch],
                                      in_=xsrc(r0, lo, hi, W + 2))
                return xa, xr

            xband0 = load_xband(0, 16)

            # small constants next (before the bulky w2 tiles hog the queues)
            muwbuf = singles.tile([128, 2, K], F16, tag="muwbuf")
            nc.sync.dma_start(out=muwbuf, in_=muw.ap())
            muw_sb = {kc: muwbuf[:, kc, :] for kc in range(2)}
            cstbuf = singles.tile([128, 2 * K + 4], F32, tag="cstbuf")
            nc.sync.dma_start(out=cstbuf, in_=cst.ap())
            bp_sb = cstbuf[:, 0:K]
            lab_sb = cstbuf[:, K:2 * K]
            cb16 = singles.tile([128, 4], F16, tag="cb16")
            nc.vector.tensor_copy(cb16, cstbuf[:, 2 * K:2 * K + 4])
            b1_sb = {mc: cb16[:, mc:mc + 1] for mc in range(2)}
            b2_sb = {mc: cb16[:, 2 + mc:3 + mc] for mc in range(2)}

            # PE warmup: keep TensorE busy through the initial DMA wait so
            # the HAM clock-gate is at 8/8 when real matmuls arrive.
            wscr = singles.tile([128, 512], F16, tag="wscr")
            nc.vector.memset(wscr, 0.0)
            with tc.tile_pool(name="psw", bufs=1, space="PSUM") as psw:
                wps = psw.tile([128, 512], F32, tag="wps", name="wps")
                for _ in range(46):
                    nc.tensor.matmul(wps, wscr[:, 0:128], wscr,
                                     start=True, stop=True)

            w2buf = singles.tile([128, 72, 128], F16, tag="w2buf")
            nc.sync.dma_start(out=w2buf[:, 0:24, :], in_=w2l.ap()[:, 0:24, :])
            nc.sync.dma_start(out=w2buf[:, 24:72, :], in_=w2l.ap()[:, 24:72, :])
            # Winograd-transformed conv2 weights: index (pos, kc, dc, mc)
            w2_sb = {(p, kc, dc, mc): w2buf[:, ((p * 2 + kc) * 3 + dc) * 2 + mc, :]
                     for p in range(6) for kc in range(2)
                     for dc in range(3) for mc in range(2)}

            with tc.tile_pool(name="ps1", bufs=2, space="PSUM") as ps1, \
                 tc.tile_pool(name="ps2", bufs=5, space="PSUM") as ps2, \
                 tc.tile_pool(name="psl", bufs=1, space="PSUM") as psl:

                # Logits matmuls of ipair p are deferred and interleaved
                # into the next ipair's conv2 stream: their ~97ns LDWEIGHTS
                # then hides under the N=512 conv2 streams (the LDW port has
                # 216-2*97 ns of slack per conv2 MM) instead of serializing.
                pend = {'mms': [], 'fin': None}

                def emit_pending_mms(k):
                    while k > 0 and pend['mms']:
                        pend['mms'].pop(0)()
                        k -= 1

                def flush_pending():
                    while pend['mms']:
                        pend['mms'].pop(0)()
                    if pend['fin'] is not None:
                        pend['fin']()
                        pend['fin'] = None

                # ---- bands: band 0 split in half so the first x halo
                # load (and hence conv1) is ready sooner ------------------
                BANDS = [(0, 16), (16, 16), (32, 32), (64, 32), (96, 16), (112, 16)]
                for bi, (r0, rb) in enumerate(BANDS):
                    # x halo in two packings:
                    #  xa: p0-63 = xh rows, p64-127 = same shifted +1 col
                    #  xr: p0-63 = xh rows, p64-127 = same shifted +1 row
                    xa, xr = xband0 if bi == 0 else load_xband(r0, rb)

                    # q1 band buffer: slot j = q1 row (r0 - 1 + j), cols
                    # 1..256 real, cols 0/257 replicate pads.
                    q1b = {}
                    nslot = ((rb + 2 + 3) // 4) * 4
                    for kc in range(2):
                        q1b[kc] = q1pool.tile([128, nslot, W + 2], F16,
                                              tag=f"q1_{kc}", name=f"q1_{kc}")
                    q1v = {kc: q1b[kc].rearrange("p (s four) c -> p s four c", four=4)
                           for kc in range(2)}

                    # conv1: q1 slot j needs xh local rows j+dr (pairs), and
                    # taps (0,2),(1,2) from xr row j, tap (2,2) from xa row j+2.
                    nch = rb // 8
                    if bi == 0:
                        groups1 = [(j, 2) for j in range(1, rb + 1, 2)] + [(rb + 1, 1)]
                        ready_j = {c: 8 * c + 10 for c in range(nch - 1)}
                        ready_j[nch - 1] = rb + 1
                    else:
                        groups1 = [(j, 2) for j in range(0, rb + 2, 2)]
                        ready_j = {c: 8 * c + 9 for c in range(nch)}
                    j_to_chunk = {v: k for k, v in ready_j.items()}

                    vtiles = {}

                    def emit_vchunk(c):
                        # replicate-pad cols for the slot range this chunk reads
                        lo = (1 if bi == 0 else 0) if c == 0 else 8 * c + 2
                        hi = 8 * c + 10          # exclusive
                        for kc in range(2):
                            nc.vector.tensor_copy(
                                out=q1b[kc][:, lo:hi, 0:1],
                                in_=q1b[kc][:, lo:hi, 1:2])
                            nc.vector.tensor_copy(
                                out=q1b[kc][:, lo:hi, W + 1:W + 2],
                                in_=q1b[kc][:, lo:hi, W:W + 1])
                        if bi == 0 and c == 0:
                            for kc in range(2):
                                nc.vector.tensor_copy(
                                    out=q1b[kc][:, 0:1, :], in_=q1b[kc][:, 1:2, :])
                        # B^T row-transform (F(4,3)): 6 V planes for the two
                        # 4-row tiles i4 in {2c, 2c+1}; d_j = slot 4*i4 + j.
                        #  v0 = (d4-d2) - 4(d2-d0)      v1 = (d3+d4) - 4(d1+d2)
                        #  v2 = (d4-d3) + 4(d1-d2)      v3 = (d4-d2) + 2(d3-d1)
                        #  v4 = (d4-d2) - 2(d3-d1)      v5 = (d5-d3) - 4(d3-d1)
                        vts = {}
                        for kc in range(2):
                            dj = [q1v[kc][:, 2 * c + (j // 4):2 * c + (j // 4) + 2,
                                          j % 4, :] for j in range(6)]
                            vt = vpool.tile([128, 2, 6, W + 2], F16,
                                            tag=f"v{kc}", name=f"v{kc}")

                            def vs():
                                return vscr.tile([128, 2, W + 2], F16,
                                                 tag="vs", name="vs", bufs=8)
                            TT, TS = nc.vector.tensor_tensor, nc.vector.tensor_scalar_mul
                            # planes emitted in conv2 consumption order (v0..v5)
                            b_ = vs(); TT(b_, dj[4], dj[2], ALU.subtract)
                            h_ = vs(); TT(h_, dj[2], dj[0], ALU.subtract)
                            h4 = vs(); TS(h4, h_, 4.0)
                            TT(vt[:, :, 0, :], b_, h4, ALU.subtract)
                            f_ = vs(); TT(f_, dj[1], dj[2], ALU.add)
                            f4 = vs(); TS(f4, f_, 4.0)
                            g_ = vs(); TT(g_, dj[3], dj[4], ALU.add)
                            TT(vt[:, :, 1, :], g_, f4, ALU.subtract)
                            c_ = vs(); TT(c_, dj[1], dj[2], ALU.subtract)
                            c4 = vs(); TS(c4, c_, 4.0)
                            e_ = vs(); TT(e_, dj[4], dj[3], ALU.subtract)
                            TT(vt[:, :, 2, :], e_, c4, ALU.add)
                            a_ = vs(); TT(a_, dj[3], dj[1], ALU.subtract)
                            a2 = vs(); TS(a2, a_, 2.0)
                            TT(vt[:, :, 3, :], b_, a2, ALU.add)
                            TT(vt[:, :, 4, :], b_, a2, ALU.subtract)
                            a4 = vs(); TS(a4, a2, 2.0)
                            t_ = vs(); TT(t_, dj[5], dj[3], ALU.subtract)
                            TT(vt[:, :, 5, :], t_, a4, ALU.subtract)
                            vts[kc] = vt
                        vtiles[c] = vts

                    for j, nr in groups1:
                        for mc in range(2):
                            ps = ps1.tile([128, nr, W], F32, tag="c1ps", name="c1ps")
                            for dr in range(3):
                                nc.tensor.matmul(
                                    ps, w1a_sb[mc, dr],
                                    xa[:, j + dr:j + dr + nr, 0:W],
                                    start=(dr == 0), stop=False)
                            nc.tensor.matmul(ps, w1s_sb[mc],
                                             xa[0:64, j + 2:j + 2 + nr, 2:W + 2],
                                             start=False, stop=False)
                            nc.tensor.matmul(ps, w1r_sb[mc],
                                             xr[:, j:j + nr, 2:W + 2],
                                             start=False, stop=True)
                            nc.scalar.activation(
                                out=q1b[mc][:, j:j + nr, 1:W + 1], in_=ps,
                                func=AF.Relu, bias=b1_sb[mc], scale=1.0)
                        last_slot = j + nr - 1
                        if last_slot in j_to_chunk:
                            emit_vchunk(j_to_chunk[last_slot])

                    ob = obuf.tile([128, rb // 2, 4], F32, tag="ob", name="ob")
                    ob_flat = ob.rearrange("p g f -> p (g f)")
                    for pp in range(rb // 8):      # 8-row pairs of 4-row tiles
                        vts = vtiles[pp]
                        q2t = {}
                        for mc in range(2):
                            mcp = []
                            for q in range(6):
                                mq = ps2.tile([128, 2, W], F32, tag="m", name="m")
                                n_mm = 0
                                for kc in range(2):
                                    for dc in range(3):
                                        nc.tensor.matmul(
                                            mq, w2_sb[q, kc, dc, mc],
                                            vts[kc][:, 0:2, q, dc:dc + W],
                                            start=(n_mm == 0), stop=(n_mm == 5))
                                        n_mm += 1
                                # stage each plane to SBUF fp16 right away so
                                # the PSUM bank frees fast and the inverse
                                # runs at 2x DVE rate
                                mc_ = tupool.tile([128, 2, W], F16, tag="mcp",
                                                  name="mcp", bufs=6)
                                nc.scalar.copy(mc_, mq)
                                mcp.append(mc_)
                            # inverse A^T (F(4,3)):
                            #  out0 = m0+s+t   out1 = d+2u   out2 = s+4t
                            #  out3 = d+8u+m5  (s=m1+m2 d=m1-m2 t=m3+m4 u=m3-m4)
                            TT, TS = nc.vector.tensor_tensor, nc.vector.tensor_scalar_mul

                            def iv():
                                return tupool.tile([128, 2, W], F16, tag="ivs",
                                                   name="ivs", bufs=9)
                            s_ = iv(); TT(s_, mcp[1], mcp[2], ALU.add)
                            d_ = iv(); TT(d_, mcp[1], mcp[2], ALU.subtract)
                            t_ = iv(); TT(t_, mcp[3], mcp[4], ALU.add)
                            u_ = iv(); TT(u_, mcp[3], mcp[4], ALU.subtract)
                            q2p = q2prep.tile([128, 2, 4, W], F16,
                                              tag=f"q2p{mc}", name=f"q2p{mc}")
                            o0 = iv(); TT(o0, mcp[0], s_, ALU.add)
                            TT(q2p[:, :, 0, :], o0, t_, ALU.add)
                            u2 = iv(); TS(u2, u_, 2.0)
                            TT(q2p[:, :, 1, :], d_, u2, ALU.add)
                            t4 = iv(); TS(t4, t_, 4.0)
                            TT(q2p[:, :, 2, :], s_, t4, ALU.add)
                            u8 = iv(); TS(u8, u2, 4.0)
                            o3 = iv(); TT(o3, d_, u8, ALU.add)
                            TT(q2p[:, :, 3, :], o3, mcp[5], ALU.add)
                            q2t[mc] = q2pool.tile([128, 8, W], F16,
                                                  tag=f"q2_{mc}", name=f"q2_{mc}")
                            nc.scalar.activation(
                                out=q2t[mc], in_=q2p.rearrange("p a b c -> p (a b) c"),
                                func=AF.Relu, bias=b2_sb[mc], scale=1.0)
                        # register this pair's logits + softmax for
                        # deferred emission
                        flush_pending()
                        pl = psl.tile([128, 16, K], F32, tag="lps", name="pl")
                        qf = {kc: q2t[kc].rearrange("p a b -> p (a b)")
                              for kc in range(2)}

                        def lmm(pl=pl, qf=qf):
                            for jj in range(16):
                                for kc in range(2):
                                    yield lambda jj=jj, kc=kc: nc.tensor.matmul(
                                        pl[:, jj, :],
                                        qf[kc][:, 128 * jj:128 * (jj + 1)],
                                        muw_sb[kc],
                                        start=(kc == 0), stop=(kc == 1))
                        pend['mms'] = list(lmm())

                        def fin(pl=pl, ob_flat=ob_flat, pp=pp, r0=r0, rb=rb):
                            # ACT stages pl out of PSUM so the bank frees
                            # without waiting on the (deep) DVE queue
                            pls = smx.tile([128, 16, K], F32, tag="pls", name="pls")
                            nc.scalar.copy(pls, pl)
                            # softmax over K (free axis) + label contraction
                            lg = smx.tile([128, 16, K], F32, tag="li", name="lg")
                            nc.vector.tensor_tensor(
                                lg, pls,
                                bp_sb.unsqueeze(1).to_broadcast([128, 16, K]),
                                ALU.add)
                            mx = smx.tile([128, 16], F32, tag="mx", name="mx")
                            nc.vector.reduce_max(mx, lg, axis=AX.X)
                            ls = smx.tile([128, 16, K], F32, tag="ls", name="ls")
                            nc.vector.tensor_tensor(
                                ls, lg,
                                mx.unsqueeze(2).to_broadcast([128, 16, K]),
                                ALU.subtract)
                            ex = smx.tile([128, 16, K], F32, tag="ex", name="ex")
                            nc.scalar.activation(out=ex, in_=ls, func=AF.Exp)
                            el = smx.tile([128, 16, K], F32, tag="el", name="el")
                            nc.vector.tensor_tensor(
                                el, ex,
                                lab_sb.unsqueeze(1).to_broadcast([128, 16, K]),
                                ALU.mult)
                            ssum = smx.tile([128, 16], F32, tag="ssum", name="ssum")
                            nc.vector.reduce_sum(ssum, ex, axis=AX.X)
                            wsum = smx.tile([128, 16], F32, tag="wsum", name="wsum")
                            nc.vector.reduce_sum(wsum, el, axis=AX.X)
                            rs = smx.tile([128, 16], F32, tag="rs", name="rs")
                            nc.vector.reciprocal(rs, ssum)
                            nc.vector.tensor_tensor(
                                ob_flat[:, 16 * pp:16 * pp + 16], wsum, rs, ALU.mult)
                            if pp == rb // 8 - 1:
                                # contiguous store; host unscrambles
                                # col = 2*row + jj
                                nc.sync.dma_start(
                                    out=outd.ap()[:, r0 * 2:(r0 + rb) * 2],
                                    in_=ob_flat)
                        pend['fin'] = fin
                flush_pending()

    nc.compile()
    return nc


def prep_inputs(x, w1, b1, w2, b2, w3, b3, mu, label):
    """Full inputs -> per-core in_maps."""
    w3m = w3[:, :, 0, 0]
    muW = 2.0 * (mu @ w3m)                                   # [K, Q]
    bpv = (2.0 * (mu @ b3) - (mu * mu).sum(1)).astype(np.float32)

    def pack_w(w1f, w2f):
        w1a = np.empty((2, 3, 128, 128), np.float32)
        w1r = np.empty((2, 128, 128), np.float32)
        w1s = np.empty((2, 64, 128), np.float32)
        for mc in range(2):
            ms = slice(128 * mc, 128 * (mc + 1))
            for dr in range(3):
                w1a[mc, dr, 0:64] = w1f[ms, :, dr, 0].T
                w1a[mc, dr, 64:128] = w1f[ms, :, dr, 1].T
            w1r[mc, 0:64] = w1f[ms, :, 0, 2].T
            w1r[mc, 64:128] = w1f[ms, :, 1, 2].T
            w1s[mc] = w1f[ms, :, 2, 2].T
        # Winograd F(4,3) along H for conv2: Gw[p] = sum_dr G[p,dr] w2[..dr..]
        G = np.array([[1 / 4, 0, 0],
                      [-1 / 6, -1 / 6, -1 / 6],
                      [-1 / 6, 1 / 6, -1 / 6],
                      [1 / 24, 1 / 12, 1 / 6],
                      [1 / 24, -1 / 12, 1 / 6],
                      [0, 0, 1]], np.float32)
        gw = np.einsum('pd,ocdk->pock', G, w2f.astype(np.float32))
        w2p = np.empty((72, 128, 128), np.float32)
        for p in range(6):
            for kc in range(2):
                for dc in range(3):
                    for mc in range(2):
                        idx = ((p * 2 + kc) * 3 + dc) * 2 + mc
                        w2p[idx] = gw[p, 128 * mc:128 * (mc + 1),
                                      128 * kc:128 * (kc + 1), dc].T
        w1xp = np.concatenate([w1a.reshape(6, 128, 128), w1r], axis=0)
        return (np.ascontiguousarray(w1xp.transpose(1, 0, 2)).astype(np.float16),
                np.ascontiguousarray(w1s.transpose(1, 0, 2)).astype(np.float16),
                np.ascontiguousarray(w2p.transpose(1, 0, 2)).astype(np.float16))

    packs = {}
    packs[0] = pack_w(w1, w2)
    packs[1] = pack_w(w1[:, :, ::-1, :], w2[:, :, ::-1, :])

    muwp = np.empty((128, 2, K), np.float32)
    for kc in range(2):
        muwp[:, kc, :] = muW[:, 128 * kc:128 * (kc + 1)].T
    muwp = muwp.astype(np.float16)
    cstv = np.empty((128, 2 * K + 4), np.float32)
    cstv[:, 0:K] = bpv[None, :]
    cstv[:, K:2 * K] = label[None, :].astype(np.float32)
    for mc in range(2):
        cstv[:, 2 * K + mc] = b1[128 * mc:128 * (mc + 1)]
        cstv[:, 2 * K + 2 + mc] = b2[128 * mc:128 * (mc + 1)]

    rows = np.clip(np.arange(132) - 2, 0, H - 1)
    cols = np.clip(np.arange(W + 2) - 1, 0, W - 1)
    in_maps = []
    for core in range(NCORES):
        img, half = core // 2, core % 2
        xl = x[img] if half == 0 else x[img, :, ::-1, :]
        xhv = np.ascontiguousarray(xl[:, rows][:, :, cols]).astype(np.float16)
        xhf = np.concatenate([xhv.reshape(-1),
                              np.zeros(2 * (W + 2), np.float16)])
        w1xp, w1sp, w2p = packs[half]
        in_maps.append({
            'xh': xhf, 'w1x': w1xp, 'w1s': w1sp, 'w2l': w2p,
            'muw': muwp, 'cst': cstv,
        })
    return in_maps


def gather(results, dtype=np.float32):
    out = np.empty((B, 1, H, W), dtype)
    for core in range(NCORES):
        img, half = core // 2, core % 2
        o = results[core]['out']        # [128 p, 2*row + jj]
        o = o.reshape(128, 128, 2).transpose(1, 2, 0).reshape(128, W)
        if half == 0:
            out[img, 0, 0:128] = o
        else:
            out[img, 0, 128:256] = o[::-1]
    return out


def get_nc():
    if 'nc' not in _cached:
        _cached['nc'] = build_nc()
    return _cached['nc']


def kernel(x, w1, b1, w2, b2, w3, b3, mu, label, **run_kwargs):
    nc = get_nc()
    in_maps = prep_inputs(
        np.asarray(x, np.float32), np.asarray(w1, np.float32),
        np.asarray(b1, np.float32), np.asarray(w2, np.float32),
        np.asarray(b2, np.float32), np.asarray(w3, np.float32),
        np.asarray(b3, np.float32), np.asarray(mu, np.float32),
        np.asarray(label, np.float32))
    res = run_bass_kernel_spmd(nc, in_maps, core_ids=list(range(NCORES)),
                               **run_kwargs)
    out = gather(res.results)
    if run_kwargs:
        _cached['last_result'] = res
    return out
